# revision 69
# baseline (speedup 1.0000x reference)
"""ConvLinformer forward on 8 Trainium2 NeuronCores (Bass/Tile).

Sharding: 8-way over sequence (512 tokens/core/batch). Weights replicated,
except the conv kernels [O,C,S] which are channel(C)-sliced per core; the
conv contraction is channel-sharded via an AllToAll of the ke/ve activations
followed by an AllReduce of the k_/v_ partials (the Linformer layers use the
same AllReduce for their sequence-projection partials).

Layout: residual stream kept feature-major in SBUF: x^T = [128, (dt:8, b:2,
tl:512)] (partition = feature-within-tile). Matmuls in fp32r (FP22 multiply,
fp32 accumulate, full PE rate at free>=256); conv + FFN-w2 paths in bf16.

Schedule: every phase is emitted per batch b (the two 512-token chunks ARE
the two batches), with one AllReduce per batch, so each engine stream always
has the other batch's work queued behind any collective/latency wait:
  S1(b) = ln1 -> q/k/v proj -> kv partials -> AllReduce(b)   [b = 0, 1]
  S2    = [attn -> wo -> ln2](b=0,1) -> ffn-h(b=0,1) -> ffn-w2(b=0,1)
Attention computes scores transposed (kvpos on partitions) so no PE
transposes are needed; softmax skips max-subtraction (f32 exp is range-safe
here) and folds the normalization into the PSUM writeback via a broadcast
ones-matmul. Elementwise work is spread across DVE/Act/Pool (Pool cannot
read PSUM). SBUF pools use the queue (ring) allocator to survive the
pipelined, non-LIFO pool lifetimes.

Self-contained: shapes hardcoded; host shards inputs / gathers outputs.
"""

import contextlib

import numpy as np

import concourse.bacc as bacc
import concourse.mybir as mybir
import concourse.tile as tile
from concourse.bass_utils import run_bass_kernel_spmd
from concourse.masks import make_identity

P = 128
B, N, D, H, DH, K, S, DFF, L = 2, 4096, 1024, 8, 128, 256, 16, 4096, 2
NC = 8
NL = N // NC          # 512 local tokens per batch
T = B * NL            # 1024 local tokens, free layout (b, tl)
DT = D // P           # 8 feature tiles
DFT = DFF // P        # 32 dff tiles
KT = K // P           # 2 kv-position tiles
SCALE = float(DH) ** -0.5

F32 = mybir.dt.float32
F32R = mybir.dt.float32r
BF16 = mybir.dt.bfloat16
AX = mybir.AxisListType
OP = mybir.AluOpType
AF = mybir.ActivationFunctionType

PARAM_NAMES = [
    "ln1_g", "ln1_b", "wq", "wk", "wv", "pk", "pv", "wo", "bo",
    "ln2_g", "ln2_b", "w1", "b1", "w2", "b2",
]


def _declare_io(nc):
    d = {}
    d["x_local"] = nc.dram_tensor("x_local", [B, NL, D], F32, kind="ExternalInput").ap()
    for kind in ("lin", "conv"):
        for li in range(L):
            pre = f"{kind}{li}_"
            for v in ("ln1_g", "ln1_b", "bo", "ln2_g", "ln2_b", "b2"):
                d[pre + v] = nc.dram_tensor(pre + v, [D], F32, kind="ExternalInput").ap()
            d[pre + "b1"] = nc.dram_tensor(pre + "b1", [DFF], F32, kind="ExternalInput").ap()
            for w in ("wq", "wk", "wv", "wo"):
                d[pre + w] = nc.dram_tensor(pre + w, [D, D], F32, kind="ExternalInput").ap()
            d[pre + "w1"] = nc.dram_tensor(pre + "w1", [D, DFF], F32, kind="ExternalInput").ap()
            d[pre + "w2"] = nc.dram_tensor(pre + "w2", [DFF, D], F32, kind="ExternalInput").ap()
            if kind == "lin":
                d[pre + "pk"] = nc.dram_tensor(pre + "pk", [NL, K], F32, kind="ExternalInput").ap()
                d[pre + "pv"] = nc.dram_tensor(pre + "pv", [NL, K], F32, kind="ExternalInput").ap()
            else:
                # channel-sliced conv kernels: [O, 128(c-slice), S]
                d[pre + "pk"] = nc.dram_tensor(pre + "pk", [D, P, S], F32, kind="ExternalInput").ap()
                d[pre + "pv"] = nc.dram_tensor(pre + "pv", [D, P, S], F32, kind="ExternalInput").ap()
    d["y_local"] = nc.dram_tensor("y_local", [B, NL, D], F32, kind="ExternalOutput").ap()
    return d


class Ctx:
    def __init__(self, nc, tc, io):
        self.nc, self.tc, self.io = nc, tc, io
        self.single_core = False
        self.sections = []

    def mark(self, name):
        # probe the instruction-id counter (consumes one id) for profiling
        self.sections.append((name, self.nc.next_id()))

    def collective(self, kind, op, ins, outs):
        nc = self.nc
        if self.single_core:
            # timing-sim mode: stand in a local DRAM->DRAM copy for the
            # collective so TimelineSim (single-core) can schedule it.
            nc.sync.dma_start(outs[0], ins[0])
        else:
            nc.gpsimd.collective_compute(
                kind, op, replica_groups=[list(range(NC))], ins=ins, outs=outs)


def _load_col(ctx, dram_vec, width, pool, name):
    """Load a [width*128] dram vector as a [128, width] column tile (f32)."""
    nc = ctx.nc
    t = pool.tile([P, width], F32, name=name)
    nc.sync.dma_start(t[:], dram_vec.rearrange("(w p) -> p w", p=P))
    return t


def _layernorm(ctx, x, g_col, b_col, xn, pfx, cs=(0, 1)):
    """xn = LN(x) * g + b for token chunks in cs, feature-major f32r.

    Rows via PE ones-matmuls; rstd via Sqrt+reciprocal; final scale+bias
    fused into an Activation-engine Identity; the add on the Pool engine."""
    nc, tc = ctx.nc, ctx.tc
    with (
        tc.tile_pool(name=pfx + "sb", bufs=1) as sb,
        tc.tile_pool(name=pfx + "xq", bufs=1) as xqp,
        tc.tile_pool(name=pfx + "ps", bufs=2, space="PSUM") as ps,
        tc.tile_pool(name=pfx + "bps", bufs=1, space="PSUM") as bps,
    ):
        for ci, c in enumerate(cs):  # token chunks of 512 (c == batch)
            st1 = ps.tile([1, 512], F32, tag="st")
            st2 = ps.tile([1, 512], F32, tag="st")
            for dt in range(DT):
                nc.tensor.matmul(st1[:], ctx.ones_col[:], x[:, dt * T + c * 512:][:, :512],
                                 start=(dt == 0), stop=(dt == DT - 1))
            for dt in range(DT):
                # per-dt squares so this LN can chase the producer of x
                xv = x[:, dt * T + c * 512:][:, :512]
                xsq = xqp.tile([P, 512], F32R, tag="xsq", bufs=2)
                nc.vector.tensor_mul(xsq[:], xv.bitcast(F32), xv.bitcast(F32))
                nc.tensor.matmul(st2[:], ctx.ones_col[:], xsq[:],
                                 start=(dt == 0), stop=(dt == DT - 1))
            m_row = sb.tile([1, 512], F32, tag="m")
            nc.vector.tensor_scalar_mul(m_row[:], st1[:], 1.0 / D)
            msq = sb.tile([1, 512], F32, tag="msq")
            nc.vector.tensor_mul(msq[:], m_row[:], m_row[:])
            var = sb.tile([1, 512], F32, tag="var")
            nc.vector.scalar_tensor_tensor(var[:], st2[:], 1.0 / D, msq[:], OP.mult, OP.subtract)
            sd = sb.tile([1, 512], F32, tag="sd")
            nc.scalar.activation(sd[:], var[:], AF.Sqrt, bias=ctx.eps_b[:], scale=1.0)
            r_row = sb.tile([1, 512], F32R, tag="r")
            with nc.allow_low_precision(reason="ln rstd row -> f32r bcast rhs"):
                nc.vector.reciprocal(r_row[:], sd[:])
            s0_row = sb.tile([1, 512], F32R, tag="s0r")
            with nc.allow_low_precision(reason="ln -m*r row -> f32r bcast rhs"):
                nc.vector.scalar_tensor_tensor(
                    s0_row[:], m_row[:], -1.0, r_row[:], OP.mult, OP.mult)
            s0ps = bps.tile([P, 512], F32, tag="bc0")
            s1ps = bps.tile([P, 512], F32, tag="bc1")
            nc.tensor.matmul(s0ps[:], ctx.ones_row[:], s0_row[:], start=True, stop=True)
            nc.tensor.matmul(s1ps[:], ctx.ones_row[:], r_row[:], start=True, stop=True)
            s0bc = sb.tile([P, 512], F32, tag="s0bc")
            s1bc = sb.tile([P, 512], F32, tag="s1bc")
            nc.scalar.activation(s0bc[:], s0ps[:], AF.Copy)
            nc.vector.tensor_copy(s1bc[:], s1ps[:])
            for dt in range(DT):
                sl = slice(dt * T + c * 512, dt * T + c * 512 + 512)
                p1 = sb.tile([P, 512], F32, tag="p1", bufs=2)
                nc.vector.tensor_mul(p1[:], x[:, sl].bitcast(F32), s1bc[:])
                p2 = sb.tile([P, 512], F32, tag="p2", bufs=2)
                nc.gpsimd.tensor_add(p2[:], p1[:], s0bc[:])
                nc.scalar.activation(xn[:, sl], p2[:], AF.Identity,
                                     bias=b_col[:, dt:dt + 1], scale=g_col[:, dt:dt + 1])


def _proj_T(ctx, w_dram, src, out_cb, pfx, cs=(0, 1)):
    """Feature-major projection for token chunks in cs: psum[ot, c] =
    sum_dt W[dt,ot].T @ src[dt,c]. Weights streamed per call."""
    nc, tc = ctx.nc, ctx.tc
    with (
        tc.tile_pool(name=pfx + "w", bufs=DT) as wp,
        tc.tile_pool(name=pfx + "ps", bufs=3, space="PSUM") as ps,
    ):
        w_sb = []
        for dt in range(DT):
            wt = wp.tile([P, D], F32R, tag="w", name=f"w{dt}")
            nc.sync.dma_start(wt[:], w_dram[dt * P:(dt + 1) * P, :].bitcast(F32R))
            w_sb.append(wt)
        for c in cs:
            for ot in range(DT):
                pp = ps.tile([P, 512], F32, tag="pj")
                for dt in range(DT):
                    nc.tensor.matmul(pp[:], w_sb[dt][:, ot * P:(ot + 1) * P],
                                     src[:, dt * T + c * 512:][:, :512],
                                     start=(dt == 0), stop=(dt == DT - 1))
                out_cb(ot, c, pp)


def _lin_kv_c(ctx, li, xn, cc_in_b, b, pfx):
    """Linformer kv partials for batch b -> cc_in_b [128, 4096]
    (k^T dt-major in [0:2048], v token-major in [2048:4096])."""
    nc, tc = ctx.nc, ctx.tc
    io = ctx.io
    pre = f"lin{li}_"
    with (
        tc.tile_pool(name=pfx + "w", bufs=DT) as wp,
        tc.tile_pool(name=pfx + "kv", bufs=2) as kvp,
        tc.tile_pool(name=pfx + "p", bufs=1) as pp_,
        tc.tile_pool(name=pfx + "ar", bufs=1) as arp,
        tc.tile_pool(name=pfx + "ps", bufs=4, space="PSUM") as ps,
        tc.tile_pool(name=pfx + "ps2", bufs=2, space="PSUM") as ps2,
    ):
        pk_sb = pp_.tile([P, 4 * K], F32R, name="pk_sb")
        nc.sync.dma_start(pk_sb[:].rearrange("p (nt k) -> p nt k", nt=4),
                          io[pre + "pk"].rearrange("(nt p) k -> p nt k", p=P).bitcast(F32R))
        pv_sb = pp_.tile([P, 4 * K], F32R, name="pv_sb")
        nc.sync.dma_start(pv_sb[:].rearrange("p (nt k) -> p nt k", nt=4),
                          io[pre + "pv"].rearrange("(nt p) k -> p nt k", p=P).bitcast(F32R))
        arh = arp.tile([P, 4096], F32, tag="arh", name="arh")

        for ten in range(2):  # 0 = k, 1 = v
            wname = pre + ("wk" if ten == 0 else "wv")
            w_sb = []
            for dt in range(DT):
                wt = wp.tile([P, D], F32R, tag="w", name=f"w{dt}")
                nc.sync.dma_start(wt[:], io[wname][dt * P:(dt + 1) * P, :].bitcast(F32R))
                w_sb.append(wt)
            full = kvp.tile([P, 4 * D], F32R, tag="full", name="full")
            for nt in range(4):
                for c2 in range(2):
                    fp = ps.tile([P, 512], F32, tag="pf")
                    for dt in range(DT):
                        lhs = xn[:, dt * T + b * 512 + nt * P:][:, :P]
                        nc.tensor.matmul(fp[:], lhs, w_sb[dt][:, c2 * 512:][:, :512],
                                         start=(dt == 0), stop=(dt == DT - 1))
                    if (nt * 2 + c2) % 2 == 0:
                        nc.scalar.activation(full[:, nt * D + c2 * 512:][:, :512], fp[:], AF.Copy)
                    else:
                        nc.vector.tensor_copy(full[:, nt * D + c2 * 512:][:, :512], fp[:])
            if ten == 0:
                # k_^T partials: [dt][128, K]
                for dt in range(DT):
                    kp = ps2.tile([P, K], F32, tag="kp")
                    for nt in range(4):
                        nc.tensor.matmul(kp[:], full[:, nt * D + dt * P:][:, :P],
                                         pk_sb[:, nt * K:][:, :K],
                                         start=(nt == 0), stop=(nt == 3))
                    if dt % 2 == 0:
                        nc.scalar.activation(arh[:, dt * K:][:, :K], kp[:], AF.Copy)
                    else:
                        nc.vector.tensor_copy(arh[:, dt * K:][:, :K], kp[:])
            else:
                # v_ token-major partials: [kt][128, D]
                for kt in range(KT):
                    for c2 in range(2):
                        vp = ps2.tile([P, 512], F32, tag="vp")
                        for nt in range(4):
                            nc.tensor.matmul(vp[:], pv_sb[:, nt * K + kt * P:][:, :P],
                                             full[:, nt * D + c2 * 512:][:, :512],
                                             start=(nt == 0), stop=(nt == 3))
                        if (kt * 2 + c2) % 2 == 0:
                            nc.scalar.activation(
                                arh[:, 2048 + kt * D + c2 * 512:][:, :512], vp[:], AF.Copy)
                        else:
                            nc.vector.tensor_copy(
                                arh[:, 2048 + kt * D + c2 * 512:][:, :512], vp[:])
        nc.sync.dma_start(cc_in_b[:], arh[:])


def _conv_wt(ctx, li, ten, wtp, wnp, wps, pfx):
    """Transpose one channel-sliced conv kernel [O,128c,S] -> [c, (s, o)] bf16.
    Same pool tag both calls, so the second build reuses the first's space."""
    nc, tc = ctx.nc, ctx.tc
    io = ctx.io
    pre = f"conv{li}_"
    wname = pre + ("pk" if ten == 0 else "pv")
    wt_sb = wtp.tile([P, S * D], BF16, tag="wt", name=f"wt{ten}")
    for ot in range(DT):
        wn = wnp.tile([P, P * S], F32R, tag="wn")
        nc.sync.dma_start(
            wn[:], io[wname][ot * P:(ot + 1) * P].rearrange("o c s -> o (c s)").bitcast(F32R))
        for s4 in range(4):
            tp_ps = wps.tile([P, 512], F32R, tag="wtp")
            for si in range(4):
                s = s4 * 4 + si
                nc.tensor.transpose(
                    tp_ps[:, si * P:(si + 1) * P],
                    wn[:].rearrange("o (c s) -> o s c", s=S)[:, s],
                    ctx.ident_r[:])
            dst = wt_sb[:].rearrange("c (s o) -> c s o", s=S)[:, s4 * 4:(s4 + 1) * 4,
                                                             ot * P:(ot + 1) * P]
            srcv = tp_ps[:].rearrange("c (si o) -> c si o", si=4).bitcast(F32)
            if s4 % 2 == 0:
                nc.vector.tensor_copy(dst, srcv)
            else:
                nc.scalar.activation(dst, srcv, AF.Copy)
    return wt_sb


def _conv_readback1(ctx, a2a_out_b, csp, ten, b, pfx):
    """Pull this core's channel slice of ke or ve for batch b: [128c, (peer, tl)]."""
    nc = ctx.nc
    cs = csp.tile([P, N], BF16, tag=f"cs{ten}{b}", name=f"cs{ten}{b}")
    nc.sync.dma_start(
        cs[:].rearrange("c (j t) -> c j t", j=NC),
        a2a_out_b[:, ten].rearrange("j c t -> c j t"))
    return cs


def _conv_k_c(ctx, ecs, wt_sb, arh, cps, pfx):
    """k_^T feature-major partials for one batch: [ot][128, K] -> arh[0:2048]."""
    nc = ctx.nc
    for ot in range(DT):
        kp = cps.tile([P, K], F32, tag="ck")
        for s in range(S):
            rhs = ecs[:].rearrange("c (j t) -> c j t", j=NC)[:, :, s::S]
            nc.tensor.matmul(kp[:].rearrange("o (j w) -> o j w", j=NC),
                             wt_sb[:, s * D + ot * P:][:, :P], rhs,
                             start=(s == 0), stop=(s == S - 1))
        if ot % 2 == 0:
            nc.scalar.activation(arh[:, ot * K:][:, :K], kp[:], AF.Copy)
        else:
            nc.vector.tensor_copy(arh[:, ot * K:][:, :K], kp[:])


def _conv_v_c(ctx, ecs, wt_sb, arh, cps, pfx):
    """v_ token-major partials for one batch: [kt][128, D] -> arh[2048:]."""
    nc = ctx.nc
    for kt in range(KT):
        for c2 in range(2):
            vp = cps.tile([P, 512], F32, tag="cv")
            for s in range(S):
                lhs = ecs[:].rearrange(
                    "c (j t) -> c j t", j=NC)[:, kt * 4:(kt + 1) * 4, s::S]
                nc.tensor.matmul(vp[:], lhs,
                                 wt_sb[:, s * D + c2 * 512:][:, :512],
                                 start=(s == 0), stop=(s == S - 1))
            if (kt * 2 + c2) % 2 == 0:
                nc.scalar.activation(
                    arh[:, 2048 + kt * D + c2 * 512:][:, :512], vp[:], AF.Copy)
            else:
                nc.vector.tensor_copy(
                    arh[:, 2048 + kt * D + c2 * 512:][:, :512], vp[:])


def _attention_c(ctx, qo_sb, kv_sb_b, b, pfx):
    """Per h for batch b: scores^T (kvpos on partitions), exp w/o max-sub,
    denominator via ones-matmul, normalization fused into writeback."""
    nc, tc = ctx.nc, ctx.tc
    with (
        tc.tile_pool(name=pfx + "ae", bufs=2) as aep,
        tc.tile_pool(name=pfx + "rw", bufs=2) as rwp,
        tc.tile_pool(name=pfx + "ps", bufs=2, space="PSUM") as ps_s,
        tc.tile_pool(name=pfx + "pd", bufs=1, space="PSUM") as ps_d,
        tc.tile_pool(name=pfx + "po", bufs=2, space="PSUM") as ps_o,
    ):
        for h in range(H):
            a_sb = aep.tile([P, KT * 512], F32R, tag="ae", name="a_sb")
            sc_ps = ps_s.tile([P, KT * 512], F32, tag="sc")
            for kt in range(KT):
                nc.tensor.matmul(sc_ps[:, kt * 512:][:, :512],
                                 kv_sb_b[:, h * K + kt * P:][:, :P],
                                 qo_sb[:, h * T + b * 512:][:, :512],
                                 start=True, stop=True)
                nc.scalar.activation(a_sb[:, kt * 512:][:, :512],
                                     sc_ps[:, kt * 512:][:, :512],
                                     AF.Exp, scale=SCALE)
            den_ps = ps_d.tile([1, 512], F32, tag="den")
            for kt in range(KT):
                nc.tensor.matmul(den_ps[:], ctx.ones_col[:],
                                 a_sb[:, kt * 512:][:, :512],
                                 start=(kt == 0), stop=(kt == KT - 1))
            inv_row = rwp.tile([1, 512], F32R, tag="inv")
            with nc.allow_low_precision(reason="softmax denom reciprocal -> f32r rhs"):
                nc.vector.reciprocal(inv_row[:], den_ps[:])
            dbc_ps = ps_d.tile([P, 512], F32, tag="dbc")
            nc.tensor.matmul(dbc_ps[:], ctx.ones_row[:], inv_row[:],
                             start=True, stop=True)
            dbc_sb = rwp.tile([P, 512], F32, tag="dbcs")
            nc.scalar.activation(dbc_sb[:], dbc_ps[:], AF.Copy)
            oo = ps_o.tile([P, 512], F32, tag="oo")
            for kt in range(KT):
                nc.tensor.matmul(oo[:],
                                 kv_sb_b[:, 2048 + kt * D + h * P:][:, :P],
                                 a_sb[:, kt * 512:][:, :512],
                                 start=(kt == 0), stop=(kt == KT - 1))
            nc.vector.tensor_mul(qo_sb[:, h * T + b * 512:][:, :512],
                                 oo[:], dbc_sb[:])


def _ffn_h_both(ctx, pre, xn2, b1_col, hp, pfx):
    """FFN stage A for both batches: h_b = gelu(xn2_b @ w1 + b1), bf16.
    w1 streamed once per fc chunk, used by both batches."""
    nc, tc = ctx.nc, ctx.tc
    io = ctx.io
    with (
        tc.tile_pool(name=pfx + "w1", bufs=DT + 1) as w1p,
        tc.tile_pool(name=pfx + "ph", bufs=4, space="PSUM") as ps_h,
    ):
        h_sb = [hp.tile([P, DFT * 512], BF16, tag=f"h{b}", name=f"h_sb{b}")
                for b in range(B)]
        for fc in range(8):
            w1_t = []
            for dt in range(DT):
                wt = w1p.tile([P, 512], F32R, tag="w1")
                nc.sync.dma_start(
                    wt[:], io[pre + "w1"][dt * P:(dt + 1) * P, fc * 512:(fc + 1) * 512].bitcast(F32R))
                w1_t.append(wt)
            for b in range(B):
                for fi in range(4):
                    ft = fc * 4 + fi
                    hh = ps_h.tile([P, 512], F32, tag="hh")
                    for dt in range(DT):
                        nc.tensor.matmul(hh[:], w1_t[dt][:, fi * P:(fi + 1) * P],
                                         xn2[:, dt * T + b * 512:][:, :512],
                                         start=(dt == 0), stop=(dt == DT - 1))
                    nc.scalar.activation(h_sb[b][:, ft * 512:(ft + 1) * 512], hh[:],
                                         AF.Gelu, bias=b1_col[:, ft:ft + 1], scale=1.0)
    return h_sb


def _ffn_o_both(ctx, pre, x, h_sb, b2_col, pfx):
    """FFN stage B for both batches: x += h_b @ w2 + b2. w2 streamed once
    per ot-slice, converted to bf16 on the Pool engine."""
    nc, tc = ctx.nc, ctx.tc
    io = ctx.io
    with (
        tc.tile_pool(name=pfx + "w2", bufs=2) as w2p,
        tc.tile_pool(name=pfx + "pf", bufs=2, space="PSUM") as ps_f,
    ):
        for ot in range(DT):
            w2s = w2p.tile([P, DFT * P], BF16, tag="w2s")
            for hf in range(2):
                w2f = w2p.tile([P, DFT * P // 2], F32, tag="w2f")
                nc.sync.dma_start(
                    w2f[:].rearrange("p (ft o) -> p ft o", ft=DFT // 2),
                    io[pre + "w2"].rearrange("(ft p) d -> p ft d", p=P)[
                        :, hf * (DFT // 2):(hf + 1) * (DFT // 2), ot * P:(ot + 1) * P])
                nc.gpsimd.tensor_copy(w2s[:, hf * (DFT * P // 2):][:, :DFT * P // 2], w2f[:])
            for b in range(B):
                ff = ps_f.tile([P, 512], F32, tag="ff")
                for ft in range(DFT):
                    nc.tensor.matmul(ff[:], w2s[:, ft * P:(ft + 1) * P],
                                     h_sb[b][:, ft * 512:(ft + 1) * 512],
                                     start=(ft == 0), stop=(ft == DFT - 1))
                sl = slice(ot * T + b * 512, ot * T + b * 512 + 512)
                nc.vector.scalar_tensor_tensor(x[:, sl], ff[:],
                                               b2_col[:, ot:ot + 1], x[:, sl].bitcast(F32),
                                               OP.add, OP.add)


def _build_layer(ctx, li, kind, x):
    nc, tc = ctx.nc, ctx.tc
    io = ctx.io
    pre = f"{kind}{li}_"
    pfx = pre
    with (
        tc.tile_pool(name=pfx + "cl", bufs=1) as clp,
        tc.tile_pool(name=pfx + "wa", bufs=1) as wap,
        tc.tile_pool(name=pfx + "dram", bufs=1, space="DRAM") as dp,
    ):
        g1_col = _load_col(ctx, io[pre + "ln1_g"], DT, clp, "g1c")
        b1c_col = _load_col(ctx, io[pre + "ln1_b"], DT, clp, "b1cc")
        bo_col = _load_col(ctx, io[pre + "bo"], DT, clp, "boc")
        g2_col = _load_col(ctx, io[pre + "ln2_g"], DT, clp, "g2c")
        b2c_col = _load_col(ctx, io[pre + "ln2_b"], DT, clp, "b2cc")
        b1f_col = _load_col(ctx, io[pre + "b1"], DFT, clp, "b1f")
        b2f_col = _load_col(ctx, io[pre + "b2"], DT, clp, "b2f")

        xn = wap.tile([P, DT * T], F32R, tag="workA", name="xn")
        cc_in = [dp.tile([P, 4096], F32, tag=f"cc_in{b}", name=f"cc_in{b}")
                 for b in range(B)]
        cc_out = [dp.tile([P, 4096], F32, tag=f"cc_out{b}", name=f"cc_out{b}",
                          addr_space="Shared") for b in range(B)]
        with contextlib.ExitStack() as stack:
            qo_stack = stack.enter_context(contextlib.ExitStack())
            qop = qo_stack.enter_context(tc.tile_pool(name=pfx + "qo", bufs=1))
            qo_sb = qop.tile([P, DT * T], F32R, tag="qo", name="qo_sb")
            s1_stack = stack.enter_context(contextlib.ExitStack())
            if kind == "conv":
                wtp = s1_stack.enter_context(tc.tile_pool(name=pfx + "wt", bufs=1))
                wnp = s1_stack.enter_context(tc.tile_pool(name=pfx + "wn", bufs=1))
                wps = s1_stack.enter_context(
                    tc.tile_pool(name=pfx + "wps", bufs=2, space="PSUM"))
                a2a_in = [dp.tile([NC, 2, P, 512], BF16, tag=f"a2a_in{b}",
                                  name=f"a2a_in{b}") for b in range(B)]
                a2a_out = [dp.tile([NC, 2, P, 512], BF16, tag=f"a2a_out{b}",
                                   name=f"a2a_out{b}") for b in range(B)]
            # ---- S1(b): ln1 both b first, then per-b projections + kv + AR
            for b in range(B):
                ctx.mark(pre + f"ln1_{b}")
                _layernorm(ctx, x, g1_col, b1c_col, xn, pfx + f"ln1{b}", cs=(b,))
            def q_cb(ot, c, pp):
                if (ot * 2 + c) % 2 == 0:
                    nc.scalar.activation(qo_sb[:, ot * T + c * 512:][:, :512], pp[:], AF.Copy)
                else:
                    nc.vector.tensor_copy(qo_sb[:, ot * T + c * 512:][:, :512], pp[:])

            ctx.mark(pre + "qproj")
            _proj_T(ctx, io[pre + "wq"], xn, q_cb, pfx + "q", cs=(0, 1))
            for b in range(B):
                if kind == "lin":
                    ctx.mark(pre + f"kv_{b}")
                    _lin_kv_c(ctx, li, xn, cc_in[b], b, pfx + f"kv{b}")
                    ctx.mark(pre + f"ar_{b}")
                    ctx.collective("AllReduce", OP.add, [cc_in[b][:]], [cc_out[b][:]])
                else:
                    with tc.tile_pool(name=pfx + f"kest{b}", bufs=3) as ksp:
                        def mk_cb(ten):
                            def cb(ot, c, pp):
                                st = ksp.tile([P, 512], BF16, tag="kest", name="kest")
                                if (ot * 2 + c) % 2 == 0:
                                    nc.scalar.activation(st[:], pp[:], AF.Copy)
                                else:
                                    nc.vector.tensor_copy(st[:], pp[:])
                                nc.sync.dma_start(a2a_in[c][ot, ten], st[:])
                            return cb
                        ctx.mark(pre + f"keve_{b}")
                        _proj_T(ctx, io[pre + "wk"], xn, mk_cb(0), pfx + f"ke{b}", cs=(b,))
                        _proj_T(ctx, io[pre + "wv"], xn, mk_cb(1), pfx + f"ve{b}", cs=(b,))
                    ctx.mark(pre + f"a2a_{b}")
                    ctx.collective("AllToAll", OP.bypass, [a2a_in[b][:]], [a2a_out[b][:]])

            if kind == "conv":
                # k then v contraction, one transposed kernel + one readback
                # set resident at a time
                arp = s1_stack.enter_context(tc.tile_pool(name=pfx + "arh", bufs=1))
                cps = s1_stack.enter_context(
                    tc.tile_pool(name=pfx + "cps", bufs=3, space="PSUM"))
                arh = [arp.tile([P, 4096], F32, tag=f"arh{b}", name=f"arh{b}")
                       for b in range(B)]
                with tc.tile_pool(name=pfx + "csk", bufs=1) as cspk:
                    ke = [_conv_readback1(ctx, a2a_out[b], cspk, 0, b, pfx)
                          for b in range(B)]
                    ctx.mark(pre + "convwtk")
                    wt_k = _conv_wt(ctx, li, 0, wtp, wnp, wps, pfx + "cwk")
                    for b in range(B):
                        ctx.mark(pre + f"convk_{b}")
                        _conv_k_c(ctx, ke[b], wt_k, arh[b], cps, pfx)
                with tc.tile_pool(name=pfx + "csv", bufs=1) as cspv:
                    ve = [_conv_readback1(ctx, a2a_out[b], cspv, 1, b, pfx)
                          for b in range(B)]
                    ctx.mark(pre + "convwtv")
                    wt_v = _conv_wt(ctx, li, 1, wtp, wnp, wps, pfx + "cwv")
                    for b in range(B):
                        ctx.mark(pre + f"convv_{b}")
                        _conv_v_c(ctx, ve[b], wt_v, arh[b], cps, pfx)
                        nc.sync.dma_start(cc_in[b][:], arh[b][:])
                        ctx.mark(pre + f"ar_{b}")
                        ctx.collective("AllReduce", OP.add, [cc_in[b][:]], [cc_out[b][:]])

            s1_stack.close()  # conv pools release SBUF/PSUM before attention
            # ---- S2: attn/wo/ln2 per b, then ffn stage-As, then stage-Bs
            kvp = qo_stack.enter_context(tc.tile_pool(name=pfx + "kvp", bufs=1))
            xn2 = wap.tile([P, DT * T], F32R, tag="workA", name="xn2")

            def wo_cb(ot, c, pp):
                sl = slice(ot * T + c * 512, ot * T + c * 512 + 512)
                nc.vector.scalar_tensor_tensor(x[:, sl], pp[:],
                                               bo_col[:, ot:ot + 1], x[:, sl].bitcast(F32),
                                               OP.add, OP.add)

            for b in range(B):
                kv_sb_b = kvp.tile([P, 4096], F32R, tag="kv", name=f"kv_sb{b}")
                nc.sync.dma_start(kv_sb_b[:], cc_out[b][:].bitcast(F32R))

                ctx.mark(pre + f"attn_{b}")
                _attention_c(ctx, qo_sb, kv_sb_b, b, pfx + f"att{b}")
                ctx.mark(pre + f"wo_{b}")
                _proj_T(ctx, io[pre + "wo"], qo_sb, wo_cb, pfx + f"wo{b}", cs=(b,))
                ctx.mark(pre + f"ln2_{b}")
                _layernorm(ctx, x, g2_col, b2c_col, xn2, pfx + f"ln2{b}", cs=(b,))
            qo_stack.close()  # free qo + kv blocks for the h tiles
            hp = stack.enter_context(tc.tile_pool(name=pfx + "hp", bufs=1))
            ctx.mark(pre + "ffnh")
            h_sb = _ffn_h_both(ctx, pre, xn2, b1f_col, hp, pfx + "ffnh")
            ctx.mark(pre + "ffno")
            _ffn_o_both(ctx, pre, x, h_sb, b2f_col, pfx + "ffno")


def build_program(single_core=False, no_collectives=False):
    nc = bacc.Bacc("TRN2", target_bir_lowering=False, debug=False,
                   num_devices=(1 if single_core else NC))
    io = _declare_io(nc)
    with tile.TileContext(nc, pool_alloc_mode="queue") as tc:
        with (
            tc.tile_pool(name="cst", bufs=1) as cst,
            tc.tile_pool(name="xp", bufs=1) as xp,
        ):
            ctx = Ctx(nc, tc, io)
            ctx.single_core = single_core or no_collectives
            _build_common(ctx, cst, xp)
            ctx.mark("end")
            nc._build_sections = list(ctx.sections)
    nc.compile()
    return nc


def _build_common(ctx, cst, xp):
    nc, tc, io = ctx.nc, ctx.tc, ctx.io
    if True:
        if True:
            ident_f = cst.tile([P, P], F32, name="ident_f")
            make_identity(nc, ident_f[:])
            ctx.ident_r = cst.tile([P, P], F32R, name="ident_r")
            nc.vector.tensor_copy(ctx.ident_r[:], ident_f[:])
            oc_f = cst.tile([P, 1], F32, name="oc_f")
            nc.vector.memset(oc_f[:], 1.0)
            ctx.ones_col = cst.tile([P, 1], F32R, name="ones_col")
            nc.vector.tensor_copy(ctx.ones_col[:], oc_f[:])
            or_f = cst.tile([1, P], F32, name="or_f")
            nc.vector.memset(or_f[:], 1.0)
            ctx.ones_row = cst.tile([1, P], F32R, name="ones_row")
            nc.vector.tensor_copy(ctx.ones_row[:], or_f[:])
            ctx.eps_b = cst.tile([1, 1], F32, name="eps_b")
            nc.vector.memset(ctx.eps_b[:], 1e-5)

            # load x -> feature-major x^T
            ctx.mark("io_in")
            x = xp.tile([P, DT * T], F32R, name="x")
            with (
                tc.tile_pool(name="iop", bufs=3) as iop,
                tc.tile_pool(name="iops", bufs=2, space="PSUM") as iops,
            ):
                for tt in range(8):  # tt = b*4 + nt
                    b, nt = divmod(tt, 4)
                    xtok = iop.tile([P, D], F32R, tag="xtok")
                    nc.sync.dma_start(xtok[:], io["x_local"][b, nt * P:(nt + 1) * P, :].bitcast(F32R))
                    for dg in range(2):
                        tps = iops.tile([P, 512], F32R, tag="xt")
                        for i in range(4):
                            dt = dg * 4 + i
                            nc.tensor.transpose(tps[:, i * P:(i + 1) * P],
                                                xtok[:, dt * P:(dt + 1) * P], ctx.ident_r[:])
                        nc.vector.tensor_copy(
                            x[:].rearrange("p (dt t) -> p dt t", dt=DT)[:, dg * 4:(dg + 1) * 4,
                                                                        b * 512 + nt * P:][:, :, :P],
                            tps[:].rearrange("p (i t) -> p i t", i=4).bitcast(F32))

            for li in range(L):
                _build_layer(ctx, li, "lin", x)
            for li in range(L):
                _build_layer(ctx, li, "conv", x)

            # write out: transpose back to token-major
            ctx.mark("io_out")
            with (
                tc.tile_pool(name="oop", bufs=3) as oop,
                tc.tile_pool(name="oops", bufs=2, space="PSUM") as oops,
            ):
                for tt in range(8):
                    b, nt = divmod(tt, 4)
                    ytok = oop.tile([P, D], F32, tag="ytok")
                    for dg in range(2):
                        tps = oops.tile([P, 512], F32R, tag="yt")
                        for i in range(4):
                            dt = dg * 4 + i
                            nc.tensor.transpose(tps[:, i * P:(i + 1) * P],
                                                x[:, dt * T + b * 512 + nt * P:][:, :P],
                                                ctx.ident_r[:])
                        nc.vector.tensor_copy(ytok[:, dg * 512:(dg + 1) * 512], tps[:].bitcast(F32))
                    nc.sync.dma_start(io["y_local"][b, nt * P:(nt + 1) * P, :], ytok[:])


_PROGRAM = None


def _get_program():
    global _PROGRAM
    if _PROGRAM is None:
        _PROGRAM = build_program()
    return _PROGRAM


def _make_in_maps(inputs):
    in_maps = []
    for c in range(NC):
        m = {"x_local": np.ascontiguousarray(inputs["x"][:, c * NL:(c + 1) * NL, :], dtype=np.float32)}
        for li in range(L):
            for kind in ("lin", "conv"):
                pre = f"{kind}{li}_"
                for nm in PARAM_NAMES:
                    v = np.asarray(inputs[f"{kind}_{nm}"][li], dtype=np.float32)
                    if nm in ("pk", "pv"):
                        if kind == "lin":
                            v = np.ascontiguousarray(v[c * NL:(c + 1) * NL, :])
                        else:
                            v = np.ascontiguousarray(v[:, c * P:(c + 1) * P, :])
                    m[pre + nm] = v
        in_maps.append(m)
    return in_maps


def kernel(**inputs):
    nc = _get_program()
    in_maps = _make_in_maps(inputs)
    res = run_bass_kernel_spmd(nc, in_maps, core_ids=list(range(NC)))
    out = np.concatenate([res.results[c]["y_local"] for c in range(NC)], axis=1)
    return out.astype(np.float32)

